# revision 1
# baseline (speedup 1.0000x reference)
"""Trainium2 Bass kernel for the MIOSTONE tree model (8-core SPMD).

Strategy
--------
The two big weight matrices are block-diagonal (tree structure:
``kron(eye(n), ones(H, K*ipc))``), so the dense 772 MB of weights carry only
~5.6 MB of real data.  Host-side we extract the diagonal blocks and shard by
subtree: core ``c`` owns depth-1 node ``c`` (64 depth-3 nodes, 8 depth-2
nodes, 1 depth-1 node).  All activations live on-chip as
[feature-on-partition, batch-on-free] so layers chain without transposes.
The only cross-core coupling (root layer + batchnorm + output projection) is
handled with an HBM AllGather of the 8 per-core [64, 32] tails, after which
every core redundantly computes the tiny root/BN/projection; core 0's output
is returned.

The gate combine ``x = g*relu(z_m) + (1-g)*x_lin`` is folded into the packed
weights: with s = g/(1-g) >= 0 we can pull the scale inside the relu, so the
per-layer combine is a single tensor add in a 1/(1-g)-scaled basis.  BN is
scale-invariant up to eps (compensated via eps' = eps/(1-g)^2) and the sign
of (1-g) (folded into gamma).  A fallback "direct" mode handles degenerate
gates with one extra scaled copy per tile.

Hardware constraints shaping the emission:
- A matmul (fused fp32 LDW+MM) can encode at most ONE sync wait, so every
  matmul may depend on at most one "processor" Tile hasn't already observed
  on PE.  Therefore: all DMAs ride the single SWDGE queue (one semaphore),
  inputs arrive in two order-chained blob DMAs, all psum drains/combines run
  on the vector engine only, and a dummy matmul after depth-3 absorbs the
  second blob's queue tick before depth-2 matmuls need it.
- Matmul psum/stationary base partitions are limited to {0, 32, 64}: depth-3
  lhsT tiles are stacked 3-high (bases 0/32/64) in 128-partition blob
  columns, depth-2 packs 2 nodes per [64, 32] psum tile.
"""

import numpy as np

import concourse.bacc as bacc
import concourse.bass as bass
import concourse.mybir as mybir
import concourse.tile as tile
from bass_rust import add_dep_helper
from concourse.bass_utils import run_bass_kernel_spmd

NCORES = 8
EPS = 1e-5
F32 = mybir.dt.float32
AF = mybir.ActivationFunctionType
ALU = mybir.AluOpType

# blob 1 (dma #1): biases/misc + xt + w3   [128, N1]
C_B3 = 0          # [128, 48]  cols m:0-15 l:16-31 lc:32-47
C_B2 = 48         # [64, 12]   m:0-3 l:4-7 lc:8-11
C_B1 = 60         # [32, 3]
C_B0 = 63         # [32, 3]
C_BN = 66         # [32, 2]    gamma', beta
C_WO = 68         # [33, 2]    [Wout.T ; bout]
C_XT = 70         # 16 tiles of [32, 32] at row base 32*(t%3), col 32*t
C_W3 = C_XT + 512  # 12 col-blocks of 128; tile (br,t) at block 6*br+t//3, row base 32*(t%3)
N1 = C_W3 + 12 * 128

# blob 2 (dma #2): w2 + w1 + w0   [128, N2]
C_W2 = 0          # [128, 1024]  (( br*8 + j)*2 + s)*32
C_W1 = 1024       # [64, 256]    (br*4 + ch)*32
C_W0 = 1280       # [128, 128]   (br*2 + k)*32
N2 = C_W0 + 128


def _extract_blocks(w, n, rows, cols):
    """Diagonal blocks of block-diag matrix w: out[i] = w[i*rows:(i+1)*rows, i*cols:(i+1)*cols]."""
    s0, s1 = w.strides
    return np.lib.stride_tricks.as_strided(
        w, (n, rows, cols), (rows * s0 + cols * s1, s0, s1)
    ).copy()


def _build_module(scaled: bool, g: float, debug: bool = False) -> bass.Bass:
    """Emit the per-core SPMD Bass module (identical program on all 8 cores)."""
    nc = bacc.Bacc(num_devices=NCORES)

    in1_d = nc.dram_tensor("in1", [128, N1], F32, kind="ExternalInput")
    in2_d = nc.dram_tensor("in2", [128, N2], F32, kind="ExternalInput")
    out_d = nc.dram_tensor("out", [32, 2], F32, kind="ExternalOutput")
    dbg_d = {}
    if debug:
        for nm, shp in [("d_u3", [128, 512]), ("d_xl3", [128, 512]),
                        ("d_u2", [64, 128]), ("d_xl2", [64, 128]),
                        ("d_u1", [32, 32]), ("d_xl1", [32, 32]),
                        ("d_ccout", [512, 32]), ("d_x1f", [128, 64]),
                        ("d_xl1f", [128, 64]), ("d_x0", [32, 32])]:
            dbg_d[nm] = nc.dram_tensor(nm, shp, F32, kind="ExternalOutput")

    eps_c = EPS / (1.0 - g) ** 2 if scaled else EPS
    sub_combine = (not scaled) and g < 0.0

    with tile.TileContext(nc) as tc:
        with (
            tc.tile_pool(name="weights", bufs=1) as wp,
            tc.tile_pool(name="acts", bufs=1) as acp,
            tc.tile_pool(name="scratch", bufs=4) as sp,
            tc.tile_pool(name="small", bufs=2) as smp,
            tc.tile_pool(name="psumL", bufs=4, space="PSUM") as pL,
            tc.tile_pool(name="psumS", bufs=4, space="PSUM") as pS,
            tc.tile_pool(name="dram", bufs=1, space="DRAM") as dp,
        ):
            in1 = wp.tile([128, N1], F32, name="in1_sb")
            dma1 = nc.gpsimd.dma_start(in1[:, :], in1_d[:, :])
            in2 = wp.tile([128, N2], F32, name="in2_sb")
            dma2 = nc.gpsimd.dma_start(in2[:, :], in2_d[:, :])
            add_dep_helper(dma2.ins, dma1.ins, False, "queue order: blob1 first")

            # slices of the input blobs
            def xt_t(t):
                rb = 32 * (t % 3)
                return in1[rb : rb + 32, C_XT + 32 * t : C_XT + 32 * (t + 1)]

            def w3_t(br, t):
                rb = 32 * (t % 3)
                cb = 6 * br + t // 3
                return in1[rb : rb + 32, C_W3 + 128 * cb : C_W3 + 128 * (cb + 1)]

            def b3c(kind, t):
                return in1[:, C_B3 + 16 * kind + t : C_B3 + 16 * kind + t + 1]

            def b2c(kind, pp):
                return in1[0:64, C_B2 + 4 * kind + pp : C_B2 + 4 * kind + pp + 1]

            def b1c(kind):
                return in1[0:32, C_B1 + kind : C_B1 + kind + 1]

            def b0c(kind):
                return in1[0:32, C_B0 + kind : C_B0 + kind + 1]

            def w2s(br, j, s):
                o = ((br * 8 + j) * 2 + s) * 32
                return in2[:, C_W2 + o : C_W2 + o + 32]

            def w1s(br, ch):
                o = (br * 4 + ch) * 32
                return in2[0:64, C_W1 + o : C_W1 + o + 32]

            def w0s(br, k):
                o = (br * 2 + k) * 32
                return in2[:, C_W0 + o : C_W0 + o + 32]

            # persistent activation buffers (feature-on-partition, batch-on-free)
            u3 = acp.tile([128, 512], F32, name="u3_sb")
            xl3 = acp.tile([128, 512], F32, name="xl3_sb")
            u2 = acp.tile([64, 4, 32], F32, name="u2_sb")
            xl2 = acp.tile([64, 4, 32], F32, name="xl2_sb")
            u1 = acp.tile([32, 32], F32, name="u1_sb")
            xl1 = acp.tile([32, 32], F32, name="xl1_sb")

            def drain_relu(dst, psum, bias_col):
                # dst = max(psum + bias, 0) on DVE
                nc.vector.tensor_scalar(
                    dst, psum, bias_col, 0.0, op0=ALU.add, op1=ALU.max
                )

            def drain_lin(dst, psum, bias_col):
                nc.vector.tensor_scalar(dst, psum, bias_col, None, op0=ALU.add)

            def combine(dst, hm_t, xl_dst, psl_t, bias_col_dir):
                """dst = (+-)hm + scaled-chain; scaled mode: chain==xl already drained."""
                if scaled:
                    nc.vector.tensor_add(dst, hm_t, xl_dst)
                else:
                    xlc = sp.tile(
                        [psl_t.shape[0], 32], F32, name="xlc", tag="xlc"
                    )
                    nc.vector.tensor_scalar(
                        xlc[:, :], psl_t, 1.0 - g, bias_col_dir,
                        op0=ALU.mult, op1=ALU.add,
                    )
                    if sub_combine:
                        nc.vector.tensor_sub(dst, xlc[:, :], hm_t)
                    else:
                        nc.vector.tensor_add(dst, hm_t, xlc[:, :])

            # ---- depth-3: 16 M-tiles of 128 features (4 nodes, K=32 true) ----
            for t in range(16):
                psm = pL.tile([128, 32], F32, name="psm", tag="psL")
                psl = pL.tile([128, 32], F32, name="psl", tag="psL")
                nc.tensor.matmul(
                    psm[:, :], lhsT=w3_t(0, t), rhs=xt_t(t), start=True, stop=True
                )
                nc.tensor.matmul(
                    psl[:, :], lhsT=w3_t(1, t), rhs=xt_t(t), start=True, stop=True
                )
                hm = sp.tile([128, 32], F32, name="hm", tag="hm")
                drain_relu(hm[:, :], psm[:, :], b3c(0, t))
                xl_dst = xl3[:, t * 32 : (t + 1) * 32]
                drain_lin(xl_dst, psl[:, :], b3c(1, t))
                combine(u3[:, t * 32 : (t + 1) * 32], hm[:, :], xl_dst, psl[:, :],
                        b3c(2, t))

            # dummy matmul: absorbs blob2's queue tick onto PE before depth-2
            psd = pS.tile([32, 2], F32, name="psd", tag="psS")
            nc.tensor.matmul(
                psd[:, :], lhsT=in2[0:32, 0:32], rhs=in2[0:32, 0:2],
                start=True, stop=True,
            )

            # ---- depth-2: 4 pairs of nodes, [64, 32] psum per pair ----
            for pp in range(4):
                ps2m = pL.tile([64, 32], F32, name="ps2m", tag="psL")
                ps2l = pL.tile([64, 32], F32, name="ps2l", tag="psL")
                for jj in range(2):
                    j = 2 * pp + jj
                    for s in range(2):
                        nc.tensor.matmul(
                            ps2m[32 * jj : 32 * (jj + 1), :],
                            lhsT=w2s(0, j, s),
                            rhs=u3[:, (2 * j + s) * 32 : (2 * j + s + 1) * 32],
                            start=(s == 0), stop=(s == 1),
                        )
                    for s in range(2):
                        nc.tensor.matmul(
                            ps2l[32 * jj : 32 * (jj + 1), :],
                            lhsT=w2s(1, j, s),
                            rhs=xl3[:, (2 * j + s) * 32 : (2 * j + s + 1) * 32],
                            start=(s == 0), stop=(s == 1),
                        )
                hm2 = sp.tile([64, 32], F32, name="hm2", tag="hm")
                drain_relu(hm2[:, :], ps2m[:, :], b2c(0, pp))
                xl_dst = xl2[:, pp, :]
                drain_lin(xl_dst, ps2l[:, :], b2c(1, pp))
                combine(u2[:, pp, :], hm2[:, :], xl_dst, ps2l[:, :], b2c(2, pp))

            # ---- depth-1: this core's single node (K=256 as 4 chunks of 64) ----
            ps1m = pS.tile([32, 32], F32, name="ps1m", tag="psS")
            ps1l = pS.tile([32, 32], F32, name="ps1l", tag="psS")
            for ch in range(4):
                nc.tensor.matmul(
                    ps1m[:, :], lhsT=w1s(0, ch), rhs=u2[:, ch, :],
                    start=(ch == 0), stop=(ch == 3),
                )
            for ch in range(4):
                nc.tensor.matmul(
                    ps1l[:, :], lhsT=w1s(1, ch), rhs=xl2[:, ch, :],
                    start=(ch == 0), stop=(ch == 3),
                )
            hm1 = smp.tile([32, 32], F32, name="hm1", tag="hm1")
            drain_relu(hm1[:, :], ps1m[:, :], b1c(0))
            drain_lin(xl1[:, :], ps1l[:, :], b1c(1))
            combine(u1[:, :], hm1[:, :], xl1[:, :], ps1l[:, :], b1c(2))

            # ---- AllGather the per-core tails: [u1 ; xl1] -> [8, 2, 32, 32] ----
            cc_in = dp.tile([64, 32], F32, name="cc_in")
            cc_out = dp.tile([512, 32], F32, name="cc_out")
            nc.gpsimd.dma_start(cc_in[0:32, :], u1[:, :])
            nc.gpsimd.dma_start(cc_in[32:64, :], xl1[:, :])
            nc.gpsimd.collective_compute(
                "AllGather", ALU.bypass,
                replica_groups=[list(range(NCORES))],
                ins=[cc_in[:, :]], outs=[cc_out[:, :]],
            )
            ccv = cc_out[:, :].rearrange("(gc st o) b -> gc st o b", gc=8, st=2)
            x1f = acp.tile([128, 2, 32], F32, name="x1f")
            xl1f = acp.tile([128, 2, 32], F32, name="xl1f")
            # order-chained so the last read (x1f k=0) covers all queue ticks
            reads = []
            for st, dst in ((1, xl1f), (0, x1f)):
                for k in (1, 0):
                    # dst flattened (p=32*gg+o, b) order == src (gg, o, b) order
                    r = nc.gpsimd.dma_start(
                        dst[:, k, :],
                        ccv[4 * k : 4 * (k + 1), st, :, :],
                    )
                    if reads:
                        add_dep_helper(r.ins, reads[-1].ins, False, "gather order")
                    reads.append(r)

            # ---- depth-0 (root), replicated on every core ----
            ps0m = pS.tile([32, 32], F32, name="ps0m", tag="psS")
            ps0l = pS.tile([32, 32], F32, name="ps0l", tag="psS")
            for k in range(2):
                nc.tensor.matmul(
                    ps0m[:, :], lhsT=w0s(0, k), rhs=x1f[:, k, :],
                    start=(k == 0), stop=(k == 1),
                )
            for k in range(2):
                nc.tensor.matmul(
                    ps0l[:, :], lhsT=w0s(1, k), rhs=xl1f[:, k, :],
                    start=(k == 0), stop=(k == 1),
                )
            hm0 = smp.tile([32, 32], F32, name="hm0", tag="hm0")
            xl0 = smp.tile([32, 32], F32, name="xl0", tag="xl0")
            x0 = smp.tile([32, 32], F32, name="x0", tag="x0")
            drain_relu(hm0[:, :], ps0m[:, :], b0c(0))
            drain_lin(xl0[:, :], ps0l[:, :], b0c(1))
            combine(x0[:, :], hm0[:, :], xl0[:, :], ps0l[:, :], b0c(2))

            # ---- batchnorm over the batch (free) axis ----
            stats = smp.tile([32, 6], F32, name="stats", tag="stats")
            mv = smp.tile([32, 2], F32, name="mv", tag="mv")
            nc.vector.bn_stats(stats[:, :], x0[:, :])
            nc.vector.bn_aggr(mv[:, :], stats[:, :])
            eps_t = wp.tile([32, 1], F32, name="eps_t")
            nc.vector.memset(eps_t[:, :], eps_c)
            sq = smp.tile([32, 1], F32, name="sq", tag="sq")
            nc.scalar.activation(
                sq[:, :], mv[:, 1:2], AF.Sqrt, bias=eps_t[:, :], scale=1.0
            )
            rstd = smp.tile([32, 1], F32, name="rstd", tag="rstd")
            nc.vector.reciprocal(rstd[:, :], sq[:, :])

            aug = acp.tile([33, 32], F32, name="aug")
            nc.vector.memset(aug[32:33, :], 1.0)
            nc.vector.tensor_scalar(
                aug[0:32, :], x0[:, :], mv[:, 0:1], rstd[:, :],
                op0=ALU.subtract, op1=ALU.mult,
            )
            nc.vector.tensor_scalar(
                aug[0:32, :], aug[0:32, :],
                in1[0:32, C_BN : C_BN + 1], in1[0:32, C_BN + 1 : C_BN + 2],
                op0=ALU.mult, op1=ALU.add,
            )
            pso = pS.tile([32, 2], F32, name="pso", tag="psS")
            nc.tensor.matmul(
                pso[:, :], lhsT=aug[:, :], rhs=in1[0:33, C_WO : C_WO + 2],
                start=True, stop=True,
            )
            outt = smp.tile([32, 2], F32, name="outt", tag="outt")
            nc.vector.tensor_copy(outt[:, :], pso[:, :])
            nc.gpsimd.dma_start(out_d[:, :], outt[:, :])
            if debug:
                for ap_src, nm in [(u3[:, :], "d_u3"), (xl3[:, :], "d_xl3"),
                                   (u2[:, :, :], "d_u2"), (xl2[:, :, :], "d_xl2"),
                                   (u1[:, :], "d_u1"), (xl1[:, :], "d_xl1"),
                                   (cc_out[:, :], "d_ccout"),
                                   (x1f[:, :, :], "d_x1f"), (xl1f[:, :, :], "d_xl1f"),
                                   (x0[:, :], "d_x0")]:
                    nc.gpsimd.dma_start(dbg_d[nm][:, :], ap_src)

    nc.finalize()
    return nc


_module_cache: dict = {}


def _get_module(scaled: bool, g: float, debug: bool = False) -> bass.Bass:
    key = (scaled, round(float(g), 12), debug)
    if key not in _module_cache:
        _module_cache[key] = _build_module(scaled, g, debug)
    return _module_cache[key]


def _pack_inputs(x, Wm3, bm3, Wl3, bl3, Wm2, bm2, Wl2, bl2, Wm1, bm1, Wl1, bl1,
                 Wm0, bm0, Wl0, bl0, gate, bn_gamma, bn_beta, Wout, bout,
                 scaled, g):
    f = np.float32
    if scaled:
        aW3 = g / (1.0 - g)  # relu-branch weight factor, d3 (raw input basis)
        aW = g               # relu-branch weight factor, d2/d1/d0 (u basis)
        ab = g / (1.0 - g)   # relu-branch bias factor, all layers
        sgn1mg = 1.0 if (1.0 - g) > 0 else -1.0
    else:
        aW3 = aW = ab = abs(g)
        sgn1mg = 1.0

    bl3m = _extract_blocks(np.asarray(Wm3, f), 128, 128, 32)  # (128, 128m, 32k)
    bl3l = _extract_blocks(np.asarray(Wl3, f), 128, 128, 32)
    bl2m = _extract_blocks(np.asarray(Wm2, f), 64, 32, 256)   # (64, 32m, 256k)
    bl2l = _extract_blocks(np.asarray(Wl2, f), 64, 32, 256)
    bl1m = _extract_blocks(np.asarray(Wm1, f), 8, 32, 256)
    bl1l = _extract_blocks(np.asarray(Wl1, f), 8, 32, 256)
    Wm0 = np.asarray(Wm0, f)
    Wl0 = np.asarray(Wl0, f)
    x = np.asarray(x, f)
    bm3 = np.asarray(bm3, f); bl3 = np.asarray(bl3, f)
    bm2 = np.asarray(bm2, f); bl2 = np.asarray(bl2, f)
    bm1 = np.asarray(bm1, f); bl1 = np.asarray(bl1, f)
    bm0 = np.asarray(bm0, f); bl0 = np.asarray(bl0, f)

    # blob 2 is identical on every core except w2/w1 (per-core nodes); w0 shared
    w0blk = np.zeros((128, 128), f)
    for k in range(2):
        w0blk[:, (0 * 2 + k) * 32 : (0 * 2 + k + 1) * 32] = (
            aW * Wm0[:, 128 * k : 128 * (k + 1)]
        ).T
        w0blk[:, (1 * 2 + k) * 32 : (1 * 2 + k + 1) * 32] = Wl0[
            :, 128 * k : 128 * (k + 1)
        ].T

    in_maps = []
    for c in range(NCORES):
        in1 = np.zeros((128, N1), f)
        in2 = np.zeros((128, N2), f)
        # biases
        for t in range(16):
            T = 16 * c + t
            in1[:, C_B3 + t] = ab * bm3[128 * T : 128 * (T + 1)]
            in1[:, C_B3 + 16 + t] = bl3[128 * T : 128 * (T + 1)]
            in1[:, C_B3 + 32 + t] = (1.0 - g) * bl3[128 * T : 128 * (T + 1)]
        for pp in range(4):
            lo = 256 * c + 64 * pp
            in1[0:64, C_B2 + pp] = ab * bm2[lo : lo + 64]
            in1[0:64, C_B2 + 4 + pp] = bl2[lo : lo + 64]
            in1[0:64, C_B2 + 8 + pp] = (1.0 - g) * bl2[lo : lo + 64]
        in1[0:32, C_B1 + 0] = ab * bm1[32 * c : 32 * (c + 1)]
        in1[0:32, C_B1 + 1] = bl1[32 * c : 32 * (c + 1)]
        in1[0:32, C_B1 + 2] = (1.0 - g) * bl1[32 * c : 32 * (c + 1)]
        in1[0:32, C_B0 + 0] = ab * bm0
        in1[0:32, C_B0 + 1] = bl0
        in1[0:32, C_B0 + 2] = (1.0 - g) * bl0
        in1[0:32, C_BN] = sgn1mg * np.asarray(bn_gamma, f)
        in1[0:32, C_BN + 1] = np.asarray(bn_beta, f)
        in1[0:32, C_WO : C_WO + 2] = np.asarray(Wout, f).T
        in1[32, C_WO : C_WO + 2] = np.asarray(bout, f)
        # xt: tile t at rows 32*(t%3), cols C_XT + 32t; [k, b] = x[b, leaf]
        xc = x[:, 512 * c : 512 * (c + 1)]
        for t in range(16):
            rb = 32 * (t % 3)
            in1[rb : rb + 32, C_XT + 32 * t : C_XT + 32 * (t + 1)] = xc[
                :, 32 * t : 32 * (t + 1)
            ].T
        # w3: lhsT tile (br, t) = scaled_block[T].T at rows 32*(t%3), block col 6*br + t//3
        for t in range(16):
            T = 16 * c + t
            rb = 32 * (t % 3)
            cbm = C_W3 + 128 * (t // 3)
            cbl = C_W3 + 128 * (6 + t // 3)
            in1[rb : rb + 32, cbm : cbm + 128] = (aW3 * bl3m[T]).T
            in1[rb : rb + 32, cbl : cbl + 128] = bl3l[T].T
        # w2: lhsT (br, j, s) = block[n2][:, 128s:128(s+1)].T
        for j in range(8):
            n2 = 8 * c + j
            for s in range(2):
                o = C_W2 + ((0 * 8 + j) * 2 + s) * 32
                in2[:, o : o + 32] = (aW * bl2m[n2][:, 128 * s : 128 * (s + 1)]).T
                o = C_W2 + ((1 * 8 + j) * 2 + s) * 32
                in2[:, o : o + 32] = bl2l[n2][:, 128 * s : 128 * (s + 1)].T
        # w1: lhsT (br, ch) = block[c][:, 64ch:64(ch+1)].T  (64 rows)
        for ch in range(4):
            o = C_W1 + (0 * 4 + ch) * 32
            in2[0:64, o : o + 32] = (aW * bl1m[c][:, 64 * ch : 64 * (ch + 1)]).T
            o = C_W1 + (1 * 4 + ch) * 32
            in2[0:64, o : o + 32] = bl1l[c][:, 64 * ch : 64 * (ch + 1)].T
        in2[:, C_W0 : C_W0 + 128] = w0blk
        in_maps.append({"in1": in1, "in2": in2})
    return in_maps


def kernel(x, Wm3, bm3, Wl3, bl3, Wm2, bm2, Wl2, bl2, Wm1, bm1, Wl1, bl1,
           Wm0, bm0, Wl0, bl0, gate, bn_gamma, bn_beta, Wout, bout,
           _trace=False, _trace_kwargs=None, _debug=False):
    g = float(np.asarray(gate))
    scaled = abs(1.0 - g) > 1e-6 and (g / (1.0 - g)) >= 0.0
    nc = _get_module(scaled, g, _debug)
    in_maps = _pack_inputs(
        x, Wm3, bm3, Wl3, bl3, Wm2, bm2, Wl2, bl2, Wm1, bm1, Wl1, bl1,
        Wm0, bm0, Wl0, bl0, gate, bn_gamma, bn_beta, Wout, bout, scaled, g,
    )
    kwargs = dict(_trace_kwargs or {})
    res = run_bass_kernel_spmd(
        nc, in_maps, core_ids=list(range(NCORES)), trace=_trace, **kwargs
    )
    out = np.asarray(res.results[0]["out"], np.float32)
    if _debug:
        return out, res
    if _trace:
        return out, res
    return out

